# revision 10
# baseline (speedup 1.0000x reference)
"""Trainium2 Bass kernel for MultiHeadEdgeAwareMessagePassing.

Math restructure (exact up to ~1e-5 truncation, validated vs reference):
  logits[i,j,h] = s_q[i,h] + s_k[j,h] + w[i,j]*c1[h] + c0[h]   (valid j: w>0)
  alpha = softmax_j(logits) * w
s_q, c0 are constant over j and cancel in the softmax; bk's contribution to
s_k scales numerator and denominator equally and cancels too. With
g[j,h] = exp(h[j]@a_k[h]), a_k[h] = u_k[h] @ Wk[h-block], v0 = h@Wv^T:
  msg[i,h,:] = Num_h[i,:] / Den_h[i]
  Num_h = W1^T (g_h*v0_h) + bv_h * (W1^T g_h)
  Den_h = mask^T g_h + c1_h (W1^T g_h)
where mask=[w>0], W1=relu(w)  (exp(c1 w) ~= 1 + c1 w, |c1 w| < 0.02; the
dropped quadratic term changes the final output by ~3e-6 relative).

Sharding: destination rows i split across 8 cores (384 rows each). Each core
reads its [3072, 384] slice of w^T plus replicated h^T and the small weights.
Host-side transposes are layout prep only; all compute runs on device.
"""

import numpy as np

N = 3072
D = 256
H = 4
DH = 64
DE = 8
NCORES = 8
ISLICE = N // NCORES  # 384
NSUB = ISLICE // 128  # 3
CJT = 4               # j-tiles per chunk
NCH = N // (128 * CJT)  # 6 chunks

_cache = {}


def _build_bass():
    import concourse.bass as bass
    import concourse.tile as tile
    from concourse import bacc, mybir
    from concourse.bass import ts
    from concourse.masks import make_identity

    dt = mybir.dt
    AF = mybir.ActivationFunctionType
    OP = mybir.AluOpType

    nc = bacc.Bacc("TRN2", target_bir_lowering=False, debug=False,
                   num_devices=NCORES)

    wt_d = nc.dram_tensor("wt", [N, ISLICE], dt.float32, kind="ExternalInput")
    ht_d = nc.dram_tensor("ht", [D, N], dt.float32, kind="ExternalInput")
    hs_d = nc.dram_tensor("hs", [ISLICE, D], dt.float32, kind="ExternalInput")
    Wk_d = nc.dram_tensor("Wk", [D, D], dt.float32, kind="ExternalInput")
    WvT_d = nc.dram_tensor("WvT", [D, D], dt.float32, kind="ExternalInput")
    WoT_d = nc.dram_tensor("WoT", [D, D], dt.float32, kind="ExternalInput")
    u_d = nc.dram_tensor("u", [H, 2 * DH + DE], dt.float32, kind="ExternalInput")
    bv_d = nc.dram_tensor("bv", [D], dt.float32, kind="ExternalInput")
    bo_d = nc.dram_tensor("bo", [D], dt.float32, kind="ExternalInput")
    Wew_d = nc.dram_tensor("Wew", [H * DE, 1], dt.float32, kind="ExternalInput")
    gam_d = nc.dram_tensor("gamma", [D], dt.float32, kind="ExternalInput")
    bet_d = nc.dram_tensor("beta", [D], dt.float32, kind="ExternalInput")
    out_d = nc.dram_tensor("out", [ISLICE, D], dt.float32, kind="ExternalOutput")

    bf = dt.bfloat16
    f32 = dt.float32

    with tile.TileContext(nc) as tc:
        with (
            tc.tile_pool(name="consts", bufs=1) as consts,
            tc.tile_pool(name="wtp", bufs=2) as wtp,
            tc.tile_pool(name="elem", bufs=2) as elem,
            tc.tile_pool(name="rhsp", bufs=3) as rhsp,
            tc.tile_pool(name="gp", bufs=2) as gp,
            tc.tile_pool(name="small", bufs=4) as small,
            tc.tile_pool(name="outp", bufs=2) as outp,
            tc.tile_pool(name="acc", bufs=1, space="PSUM") as accp,
            tc.tile_pool(name="pre4", bufs=2, space="PSUM") as pre4,
            tc.tile_pool(name="presk", bufs=1, space="PSUM") as presk,
        ):
            # ---------------- constants / setup ----------------
            # h^T in bf16, cast during DMA, 8 column chunks so jt0 starts fast
            ht_sb = consts.tile([128, 2, N], bf, tag="ht")
            ht_re = ht_d.ap().rearrange("(a p) n -> p a n", p=128)
            for c in range(8):
                nc.gpsimd.dma_start(ht_sb[:, :, ts(c, N // 8)],
                                    ht_re[:, :, ts(c, N // 8)])

            rhs_wv = consts.tile([128, 2, D], bf, tag="rhswv")
            nc.gpsimd.dma_start(rhs_wv,
                                WvT_d.ap().rearrange("(a p) n -> p a n", p=128))
            Wk_sb = consts.tile([128, 2, D], bf, tag="wk")
            nc.gpsimd.dma_start(Wk_sb,
                                Wk_d.ap().rearrange("(a p) n -> p a n", p=128))
            WoT_sb = consts.tile([128, 2, D], bf, tag="wot")
            nc.gpsimd.dma_start(WoT_sb,
                                WoT_d.ap().rearrange("(a p) n -> p a n", p=128))

            U_sb = consts.tile([128, 2, H], bf, tag="U")
            nc.vector.memset(U_sb, 0.0)
            for h in range(H):
                p0 = (h * DH) % 128
                a = (h * DH) // 128
                nc.gpsimd.dma_start(
                    U_sb[p0:p0 + DH, a, h:h + 1],
                    u_d[h, DH:2 * DH].rearrange("(f o) -> f o", o=1))

            Ue_sb = consts.tile([H * DE, H], bf, tag="Ue")
            nc.vector.memset(Ue_sb, 0.0)
            for h in range(H):
                nc.gpsimd.dma_start(
                    Ue_sb[h * DE:(h + 1) * DE, h:h + 1],
                    u_d[h, 2 * DH:2 * DH + DE].rearrange("(f o) -> f o", o=1))

            Wew_sb = consts.tile([H * DE, 1], bf, tag="wew")
            nc.gpsimd.dma_start(Wew_sb, Wew_d.ap())

            ones_sb = consts.tile([1, 128], bf, tag="ones")
            nc.vector.memset(ones_sb, 1.0)

            bo_row = consts.tile([1, 256], bf, tag="borow")
            nc.gpsimd.dma_start(bo_row, bo_d.ap().rearrange("(o f) -> o f", o=1))

            ident = consts.tile([128, 128], bf, tag="ident")
            make_identity(nc, ident)

            gam_sb = consts.tile([128, D], f32, tag="gam")
            nc.gpsimd.dma_start(
                gam_sb, bass.AP(tensor=gam_d, offset=0, ap=[[0, 128], [1, D]]))
            bet_sb = consts.tile([128, D], f32, tag="bet")
            nc.gpsimd.dma_start(
                bet_sb, bass.AP(tensor=bet_d, offset=0, ap=[[0, 128], [1, D]]))
            bvb_sb = consts.tile([128, D], f32, tag="bvb")
            nc.gpsimd.dma_start(
                bvb_sb, bass.AP(tensor=bv_d, offset=0, ap=[[0, 128], [1, D]]))

            eps_sb = consts.tile([128, 1], f32, tag="eps")
            nc.vector.memset(eps_sb, 1e-5)

            # a_k^T[dm, h] = sum_r Wk[r, dm] U[r, h]
            rhs_ak = consts.tile([128, 2, H], bf, tag="rhsak")
            for b in range(2):
                ps_ak = presk.tile([128, H], f32, tag="sk4")
                for a in range(2):
                    nc.tensor.matmul(ps_ak, Wk_sb[:, a, 128 * b:128 * (b + 1)],
                                     U_sb[:, a, :], start=(a == 0), stop=(a == 1))
                nc.vector.tensor_copy(rhs_ak[:, b, :], ps_ak)

            # c1[h] broadcast to all partitions
            ps_c1 = presk.tile([1, H], f32, tag="sk4")
            nc.tensor.matmul(ps_c1, Wew_sb, Ue_sb, start=True, stop=True)
            c1row = consts.tile([1, H], bf, tag="c1row")
            nc.vector.tensor_copy(c1row, ps_c1)
            ps_c1b = presk.tile([128, H], f32, tag="sk4")
            nc.tensor.matmul(ps_c1b, ones_sb, c1row, start=True, stop=True)
            c1b = consts.tile([128, H], f32, tag="c1b")
            nc.vector.tensor_copy(c1b, ps_c1b)

            # ---------------- persistent accumulators ----------------
            # cols 0:256 = W1.gV, 256:260 = W1.g, 260:264 = mask.g
            psA = [accp.tile([128, 264], f32, tag=f"A{s}", name=f"psA{s}")
                   for s in range(NSUB)]

            # ---------------- main loop ----------------
            for ch in range(NCH):
                wt4 = wtp.tile([128, CJT, ISLICE], f32, tag="wt")
                nc.sync.dma_start(
                    wt4, wt_d[ts(ch, 128 * CJT), :].rearrange(
                        "(j p) i -> p j i", p=128))

                W1c = elem.tile([128, CJT, ISLICE], bf, tag="W1")
                nc.scalar.activation(W1c, wt4, AF.Relu)
                mskc = elem.tile([128, CJT, ISLICE], bf, tag="msk")
                nc.gpsimd.tensor_scalar(mskc, wt4, 0.0, None, op0=OP.is_gt)

                # --- preprocessing for the CJT j-tiles of this chunk ---
                ps_v4 = pre4.tile([128, CJT, 256], f32, tag="v4")
                ps_sk4 = presk.tile([128, CJT, H], f32, tag="sk4")
                for jm in range(CJT):
                    jt = ch * CJT + jm
                    for a in range(2):
                        nc.tensor.matmul(ps_v4[:, jm, :],
                                         ht_sb[:, a, ts(jt, 128)],
                                         rhs_wv[:, a, :],
                                         start=(a == 0), stop=(a == 1))
                        nc.tensor.matmul(ps_sk4[:, jm, :],
                                         ht_sb[:, a, ts(jt, 128)],
                                         rhs_ak[:, a, :],
                                         start=(a == 0), stop=(a == 1))

                # g replicated over the 64 head dims: one ACT op
                grep4 = gp.tile([128, CJT, H, DH], f32, tag="grep")
                sk_b = bass.AP(tensor=ps_sk4.tensor, offset=ps_sk4.offset,
                               ap=[ps_sk4.ap[0], ps_sk4.ap[1], ps_sk4.ap[2],
                                   [0, DH]])
                nc.scalar.activation(grep4, sk_b, AF.Exp)

                rhs4 = rhsp.tile([128, CJT, 260], bf, tag="rhsbig")
                nc.vector.tensor_tensor(
                    out=rhs4[:, :, 0:256].rearrange(
                        "p j (h d) -> p j h d", h=H),
                    in0=ps_v4.rearrange("p j (h d) -> p j h d", h=H),
                    in1=grep4, op=OP.mult)
                nc.scalar.activation(rhs4[:, :, 256:260], ps_sk4, AF.Exp)

                st = (ch == 0)
                sp = (ch == NCH - 1)
                for jm in range(CJT):
                    for s in range(NSUB):
                        sl = ts(s, 128)
                        nc.tensor.matmul(psA[s][:, 0:260], W1c[:, jm, sl],
                                         rhs4[:, jm, :], start=st, stop=sp,
                                         skip_group_check=True)
                        nc.tensor.matmul(psA[s][:, 260:264], mskc[:, jm, sl],
                                         rhs4[:, jm, 256:260], start=st, stop=sp,
                                         skip_group_check=True)

            # ---------------- epilogue ----------------
            for s in range(NSUB):
                dg = small.tile([128, H], f32, tag="dg")
                nc.vector.tensor_copy(dg, psA[s][:, 256:260])
                den = small.tile([128, H], f32, tag="den")
                nc.vector.tensor_mul(den, c1b, dg)
                nc.vector.tensor_add(den, den, psA[s][:, 260:264])
                rden = small.tile([128, H], f32, tag="rden")
                nc.vector.reciprocal(rden, den)
                dgr = small.tile([128, H], f32, tag="dgr")
                nc.vector.tensor_mul(dgr, dg, rden)

                # msg_h = psA_V,h * rden_h + bv_h * (W1.g)_h * rden_h
                m1 = outp.tile([128, D], f32, tag="m1")
                m2 = outp.tile([128, D], f32, tag="m2")
                msg = outp.tile([128, D], bf, tag="msg")
                for h in range(H):
                    hsl = slice(h * DH, (h + 1) * DH)
                    nc.vector.tensor_scalar(m1[:, hsl], psA[s][:, hsl],
                                            rden[:, h:h + 1], None, op0=OP.mult)
                    nc.vector.tensor_scalar(m2[:, hsl], bvb_sb[:, hsl],
                                            dgr[:, h:h + 1], None, op0=OP.mult)
                nc.vector.tensor_add(msg, m1, m2)

                msgT = outp.tile([128, 2, 128], bf, tag="msgT")
                for b in range(2):
                    ps_t = pre4.tile([128, 128], bf, tag="v4")
                    nc.tensor.transpose(ps_t, msg[:, ts(b, 128)], ident)
                    nc.vector.tensor_copy(msgT[:, b, :], ps_t)

                ps_o = pre4.tile([128, D], f32, tag="v4")
                nc.tensor.matmul(ps_o, msgT[:, 0, :], WoT_sb[:, 0, :],
                                 start=True, stop=False)
                nc.tensor.matmul(ps_o, msgT[:, 1, :], WoT_sb[:, 1, :],
                                 start=False, stop=False)
                nc.tensor.matmul(ps_o, ones_sb, bo_row, start=False, stop=True)

                x = outp.tile([128, D], f32, tag="x")
                hseg = outp.tile([128, D], f32, tag="hseg")
                nc.sync.dma_start(hseg, hs_d[ts(s, 128), :])
                nc.vector.tensor_add(x, ps_o, hseg)

                stats = small.tile([128, 6], f32, tag="stats")
                nc.vector.bn_stats(out=stats, in_=x)
                mv = small.tile([128, 2], f32, tag="mv")
                nc.vector.bn_aggr(out=mv, in_=stats)
                sd = small.tile([128, 1], f32, tag="sd")
                nc.scalar.activation(sd, mv[:, 1:2], AF.Sqrt, bias=eps_sb)
                rstd = small.tile([128, 1], f32, tag="rstd")
                nc.vector.reciprocal(rstd, sd)

                y = outp.tile([128, D], f32, tag="y")
                nc.vector.tensor_scalar(y, x, mv[:, 0:1], rstd,
                                        op0=OP.subtract, op1=OP.mult)
                ot = outp.tile([128, D], f32, tag="ot")
                nc.vector.tensor_mul(ot, y, gam_sb)
                nc.vector.tensor_add(ot, ot, bet_sb)
                nc.sync.dma_start(out_d[ts(s, 128), :], ot)

    nc.compile()
    return nc


def _make_in_maps(h, w, Wk, Wv, bv, We_w, u, Wo, bo, gamma, beta, **_unused):
    f = np.float32
    h = np.ascontiguousarray(h, dtype=f)
    wT = np.ascontiguousarray(np.asarray(w, dtype=f).T)
    common = {
        "ht": np.ascontiguousarray(h.T),
        "Wk": np.ascontiguousarray(Wk, dtype=f),
        "WvT": np.ascontiguousarray(np.asarray(Wv, dtype=f).T),
        "WoT": np.ascontiguousarray(np.asarray(Wo, dtype=f).T),
        "u": np.ascontiguousarray(u, dtype=f),
        "bv": np.ascontiguousarray(bv, dtype=f),
        "bo": np.ascontiguousarray(bo, dtype=f),
        "Wew": np.ascontiguousarray(We_w, dtype=f),
        "gamma": np.ascontiguousarray(gamma, dtype=f),
        "beta": np.ascontiguousarray(beta, dtype=f),
    }
    in_maps = []
    for c in range(NCORES):
        sl = slice(c * ISLICE, (c + 1) * ISLICE)
        m = dict(common)
        m["wt"] = np.ascontiguousarray(wT[:, sl])
        m["hs"] = np.ascontiguousarray(h[sl, :])
        in_maps.append(m)
    return in_maps


def kernel(**inputs):
    from concourse.bass_utils import run_bass_kernel_spmd

    if "nc" not in _cache:
        _cache["nc"] = _build_bass()
    nc = _cache["nc"]

    in_maps = _make_in_maps(**inputs)
    res = run_bass_kernel_spmd(nc, in_maps, core_ids=list(range(NCORES)))
    out = np.concatenate([r["out"] for r in res.results], axis=0)
    return np.ascontiguousarray(out, dtype=np.float32)


# revision 11
# speedup vs baseline: 2.2977x; 2.2977x over previous
"""Trainium2 Bass kernel for MultiHeadEdgeAwareMessagePassing.

Math restructure (exact up to ~1e-5 truncation, validated vs reference):
  logits[i,j,h] = s_q[i,h] + s_k[j,h] + w[i,j]*c1[h] + c0[h]   (valid j: w>0)
  alpha = softmax_j(logits) * w
s_q, c0 are constant over j and cancel in the softmax; bk's contribution to
s_k scales numerator and denominator equally and cancels too. With
g[j,h] = exp(h[j]@a_k[h]), a_k[h] = u_k[h] @ Wk[h-block], v0 = h@Wv^T:
  msg[i,h,:] = Num_h[i,:] / Den_h[i]
  Num_h = W1^T (g_h*v0_h) + bv_h * (W1^T g_h)
  Den_h = mask^T g_h + c1_h (W1^T g_h)
where mask=[w>0], W1=relu(w)  (exp(c1 w) ~= 1 + c1 w, |c1 w| < 0.02; the
dropped quadratic term changes the final output by ~3e-6 relative).

Sharding: destination rows i split across 8 cores (384 rows each). Each core
reads its [3072, 384] slice of w^T plus replicated h^T and the small weights.
Host-side transposes are layout prep only; all compute runs on device.
"""

import numpy as np

N = 3072
D = 256
H = 4
DH = 64
DE = 8
NCORES = 8
ISLICE = N // NCORES  # 384
NSUB = ISLICE // 128  # 3
CJT = 4               # j-tiles per chunk
NCH = N // (128 * CJT)  # 6 chunks

_cache = {}


def _build_bass():
    import concourse.bass as bass
    import concourse.tile as tile
    from concourse import bacc, mybir
    from concourse.bass import ts
    from concourse.masks import make_identity

    dt = mybir.dt
    AF = mybir.ActivationFunctionType
    OP = mybir.AluOpType

    nc = bacc.Bacc("TRN2", target_bir_lowering=False, debug=False,
                   num_devices=NCORES)

    wt_d = nc.dram_tensor("wt", [N, ISLICE], dt.float32, kind="ExternalInput")
    ht_d = nc.dram_tensor("ht", [D, N], dt.float32, kind="ExternalInput")
    hs_d = nc.dram_tensor("hs", [ISLICE, D], dt.float32, kind="ExternalInput")
    Wk_d = nc.dram_tensor("Wk", [D, D], dt.float32, kind="ExternalInput")
    WvT_d = nc.dram_tensor("WvT", [D, D], dt.float32, kind="ExternalInput")
    WoT_d = nc.dram_tensor("WoT", [D, D], dt.float32, kind="ExternalInput")
    u_d = nc.dram_tensor("u", [H, 2 * DH + DE], dt.float32, kind="ExternalInput")
    bv_d = nc.dram_tensor("bv", [D], dt.float32, kind="ExternalInput")
    bo_d = nc.dram_tensor("bo", [D], dt.float32, kind="ExternalInput")
    Wew_d = nc.dram_tensor("Wew", [H * DE, 1], dt.float32, kind="ExternalInput")
    gam_d = nc.dram_tensor("gamma", [D], dt.float32, kind="ExternalInput")
    bet_d = nc.dram_tensor("beta", [D], dt.float32, kind="ExternalInput")
    out_d = nc.dram_tensor("out", [ISLICE, D], dt.float32, kind="ExternalOutput")

    bf = dt.bfloat16
    f32 = dt.float32

    with tile.TileContext(nc) as tc:
        with (
            tc.tile_pool(name="consts", bufs=1) as consts,
            tc.tile_pool(name="wtp", bufs=2) as wtp,
            tc.tile_pool(name="elem", bufs=2) as elem,
            tc.tile_pool(name="rhsp", bufs=3) as rhsp,
            tc.tile_pool(name="gp", bufs=2) as gp,
            tc.tile_pool(name="small", bufs=4) as small,
            tc.tile_pool(name="outp", bufs=2) as outp,
            tc.tile_pool(name="acc", bufs=1, space="PSUM") as accp,
            tc.tile_pool(name="pre4", bufs=2, space="PSUM") as pre4,
            tc.tile_pool(name="presk", bufs=1, space="PSUM") as presk,
        ):
            # ---------------- constants / setup ----------------
            # h^T in bf16, cast during DMA, 8 column chunks so jt0 starts fast
            ht_sb = consts.tile([128, 2, N], bf, tag="ht")
            ht_re = ht_d.ap().rearrange("(a p) n -> p a n", p=128)
            for c in range(8):
                nc.gpsimd.dma_start(ht_sb[:, :, ts(c, N // 8)],
                                    ht_re[:, :, ts(c, N // 8)])

            rhs_wv = consts.tile([128, 2, D], bf, tag="rhswv")
            nc.gpsimd.dma_start(rhs_wv,
                                WvT_d.ap().rearrange("(a p) n -> p a n", p=128))
            Wk_sb = consts.tile([128, 2, D], bf, tag="wk")
            nc.gpsimd.dma_start(Wk_sb,
                                Wk_d.ap().rearrange("(a p) n -> p a n", p=128))
            WoT_sb = consts.tile([128, 2, D], bf, tag="wot")
            nc.gpsimd.dma_start(WoT_sb,
                                WoT_d.ap().rearrange("(a p) n -> p a n", p=128))

            U_sb = consts.tile([128, 2, H], bf, tag="U")
            nc.vector.memset(U_sb, 0.0)
            for h in range(H):
                p0 = (h * DH) % 128
                a = (h * DH) // 128
                nc.gpsimd.dma_start(
                    U_sb[p0:p0 + DH, a, h:h + 1],
                    u_d[h, DH:2 * DH].rearrange("(f o) -> f o", o=1))

            Ue_sb = consts.tile([H * DE, H], bf, tag="Ue")
            nc.vector.memset(Ue_sb, 0.0)
            for h in range(H):
                nc.gpsimd.dma_start(
                    Ue_sb[h * DE:(h + 1) * DE, h:h + 1],
                    u_d[h, 2 * DH:2 * DH + DE].rearrange("(f o) -> f o", o=1))

            Wew_sb = consts.tile([H * DE, 1], bf, tag="wew")
            nc.gpsimd.dma_start(Wew_sb, Wew_d.ap())

            ones_sb = consts.tile([1, 128], bf, tag="ones")
            nc.vector.memset(ones_sb, 1.0)

            bo_row = consts.tile([1, 256], bf, tag="borow")
            nc.gpsimd.dma_start(bo_row, bo_d.ap().rearrange("(o f) -> o f", o=1))

            ident = consts.tile([128, 128], bf, tag="ident")
            make_identity(nc, ident)

            gam_sb = consts.tile([128, D], f32, tag="gam")
            nc.gpsimd.dma_start(
                gam_sb, bass.AP(tensor=gam_d, offset=0, ap=[[0, 128], [1, D]]))
            bet_sb = consts.tile([128, D], f32, tag="bet")
            nc.gpsimd.dma_start(
                bet_sb, bass.AP(tensor=bet_d, offset=0, ap=[[0, 128], [1, D]]))
            bvb_sb = consts.tile([128, D], f32, tag="bvb")
            nc.gpsimd.dma_start(
                bvb_sb, bass.AP(tensor=bv_d, offset=0, ap=[[0, 128], [1, D]]))

            eps_sb = consts.tile([128, 1], f32, tag="eps")
            nc.vector.memset(eps_sb, 1e-5)

            # a_k^T[dm, h] = sum_r Wk[r, dm] U[r, h]
            rhs_ak = consts.tile([128, 2, H], bf, tag="rhsak")
            for b in range(2):
                ps_ak = presk.tile([128, H], f32, tag="sk4")
                for a in range(2):
                    nc.tensor.matmul(ps_ak, Wk_sb[:, a, 128 * b:128 * (b + 1)],
                                     U_sb[:, a, :], start=(a == 0), stop=(a == 1))
                nc.vector.tensor_copy(rhs_ak[:, b, :], ps_ak)

            # c1[h] broadcast to all partitions
            ps_c1 = presk.tile([1, H], f32, tag="sk4")
            nc.tensor.matmul(ps_c1, Wew_sb, Ue_sb, start=True, stop=True)
            c1row = consts.tile([1, H], bf, tag="c1row")
            nc.vector.tensor_copy(c1row, ps_c1)
            ps_c1b = presk.tile([128, H], f32, tag="sk4")
            nc.tensor.matmul(ps_c1b, ones_sb, c1row, start=True, stop=True)
            c1b = consts.tile([128, H], f32, tag="c1b")
            nc.vector.tensor_copy(c1b, ps_c1b)

            # ---------------- persistent accumulators ----------------
            # cols 0:256 = W1.gV, 256:260 = W1.g, 260:264 = mask.g
            psA = [accp.tile([128, 264], f32, tag=f"A{s}", name=f"psA{s}")
                   for s in range(NSUB)]

            # ---------------- main loop ----------------
            for ch in range(NCH):
                wt4 = wtp.tile([128, CJT, ISLICE], f32, tag="wt")
                nc.sync.dma_start(
                    wt4, wt_d[ts(ch, 128 * CJT), :].rearrange(
                        "(j p) i -> p j i", p=128))

                W1c = elem.tile([128, CJT, ISLICE], bf, tag="W1")
                nc.scalar.activation(W1c, wt4, AF.Relu)
                mskc = elem.tile([128, CJT, ISLICE], bf, tag="msk")
                nc.vector.tensor_scalar(mskc, wt4, 0.0, None, op0=OP.is_gt)

                # --- preprocessing for the CJT j-tiles of this chunk ---
                ps_v4 = pre4.tile([128, CJT, 256], f32, tag="v4")
                ps_sk4 = presk.tile([128, CJT, H], f32, tag="sk4")
                for jm in range(CJT):
                    jt = ch * CJT + jm
                    for a in range(2):
                        nc.tensor.matmul(ps_v4[:, jm, :],
                                         ht_sb[:, a, ts(jt, 128)],
                                         rhs_wv[:, a, :],
                                         start=(a == 0), stop=(a == 1))
                        nc.tensor.matmul(ps_sk4[:, jm, :],
                                         ht_sb[:, a, ts(jt, 128)],
                                         rhs_ak[:, a, :],
                                         start=(a == 0), stop=(a == 1))

                # g replicated over the 64 head dims: one ACT op
                grep4 = gp.tile([128, CJT, H, DH], f32, tag="grep")
                sk_b = bass.AP(tensor=ps_sk4.tensor, offset=ps_sk4.offset,
                               ap=[ps_sk4.ap[0], ps_sk4.ap[1], ps_sk4.ap[2],
                                   [0, DH]])
                nc.scalar.activation(grep4, sk_b, AF.Exp)

                rhs4 = rhsp.tile([128, CJT, 260], bf, tag="rhsbig")
                nc.vector.tensor_tensor(
                    out=rhs4[:, :, 0:256].rearrange(
                        "p j (h d) -> p j h d", h=H),
                    in0=ps_v4.rearrange("p j (h d) -> p j h d", h=H),
                    in1=grep4, op=OP.mult)
                nc.scalar.activation(rhs4[:, :, 256:260], ps_sk4, AF.Exp)

                st = (ch == 0)
                sp = (ch == NCH - 1)
                for jm in range(CJT):
                    for s in range(NSUB):
                        sl = ts(s, 128)
                        nc.tensor.matmul(psA[s][:, 0:260], W1c[:, jm, sl],
                                         rhs4[:, jm, :], start=st, stop=sp,
                                         skip_group_check=True)
                        nc.tensor.matmul(psA[s][:, 260:264], mskc[:, jm, sl],
                                         rhs4[:, jm, 256:260], start=st, stop=sp,
                                         skip_group_check=True)

            # ---------------- epilogue ----------------
            for s in range(NSUB):
                dg = small.tile([128, H], f32, tag="dg")
                nc.vector.tensor_copy(dg, psA[s][:, 256:260])
                den = small.tile([128, H], f32, tag="den")
                nc.vector.tensor_mul(den, c1b, dg)
                nc.vector.tensor_add(den, den, psA[s][:, 260:264])
                rden = small.tile([128, H], f32, tag="rden")
                nc.vector.reciprocal(rden, den)
                dgr = small.tile([128, H], f32, tag="dgr")
                nc.vector.tensor_mul(dgr, dg, rden)

                # msg_h = psA_V,h * rden_h + bv_h * (W1.g)_h * rden_h
                m1 = outp.tile([128, D], f32, tag="m1")
                m2 = outp.tile([128, D], f32, tag="m2")
                msg = outp.tile([128, D], bf, tag="msg")
                for h in range(H):
                    hsl = slice(h * DH, (h + 1) * DH)
                    nc.vector.tensor_scalar(m1[:, hsl], psA[s][:, hsl],
                                            rden[:, h:h + 1], None, op0=OP.mult)
                    nc.vector.tensor_scalar(m2[:, hsl], bvb_sb[:, hsl],
                                            dgr[:, h:h + 1], None, op0=OP.mult)
                nc.vector.tensor_add(msg, m1, m2)

                msgT = outp.tile([128, 2, 128], bf, tag="msgT")
                for b in range(2):
                    ps_t = pre4.tile([128, 128], bf, tag="v4")
                    nc.tensor.transpose(ps_t, msg[:, ts(b, 128)], ident)
                    nc.vector.tensor_copy(msgT[:, b, :], ps_t)

                ps_o = pre4.tile([128, D], f32, tag="v4")
                nc.tensor.matmul(ps_o, msgT[:, 0, :], WoT_sb[:, 0, :],
                                 start=True, stop=False)
                nc.tensor.matmul(ps_o, msgT[:, 1, :], WoT_sb[:, 1, :],
                                 start=False, stop=False)
                nc.tensor.matmul(ps_o, ones_sb, bo_row, start=False, stop=True)

                x = outp.tile([128, D], f32, tag="x")
                hseg = outp.tile([128, D], f32, tag="hseg")
                nc.sync.dma_start(hseg, hs_d[ts(s, 128), :])
                nc.vector.tensor_add(x, ps_o, hseg)

                stats = small.tile([128, 6], f32, tag="stats")
                nc.vector.bn_stats(out=stats, in_=x)
                mv = small.tile([128, 2], f32, tag="mv")
                nc.vector.bn_aggr(out=mv, in_=stats)
                sd = small.tile([128, 1], f32, tag="sd")
                nc.scalar.activation(sd, mv[:, 1:2], AF.Sqrt, bias=eps_sb)
                rstd = small.tile([128, 1], f32, tag="rstd")
                nc.vector.reciprocal(rstd, sd)

                y = outp.tile([128, D], f32, tag="y")
                nc.vector.tensor_scalar(y, x, mv[:, 0:1], rstd,
                                        op0=OP.subtract, op1=OP.mult)
                ot = outp.tile([128, D], f32, tag="ot")
                nc.vector.tensor_mul(ot, y, gam_sb)
                nc.vector.tensor_add(ot, ot, bet_sb)
                nc.sync.dma_start(out_d[ts(s, 128), :], ot)

    nc.compile()
    return nc


def _make_in_maps(h, w, Wk, Wv, bv, We_w, u, Wo, bo, gamma, beta, **_unused):
    f = np.float32
    h = np.ascontiguousarray(h, dtype=f)
    wT = np.ascontiguousarray(np.asarray(w, dtype=f).T)
    common = {
        "ht": np.ascontiguousarray(h.T),
        "Wk": np.ascontiguousarray(Wk, dtype=f),
        "WvT": np.ascontiguousarray(np.asarray(Wv, dtype=f).T),
        "WoT": np.ascontiguousarray(np.asarray(Wo, dtype=f).T),
        "u": np.ascontiguousarray(u, dtype=f),
        "bv": np.ascontiguousarray(bv, dtype=f),
        "bo": np.ascontiguousarray(bo, dtype=f),
        "Wew": np.ascontiguousarray(We_w, dtype=f),
        "gamma": np.ascontiguousarray(gamma, dtype=f),
        "beta": np.ascontiguousarray(beta, dtype=f),
    }
    in_maps = []
    for c in range(NCORES):
        sl = slice(c * ISLICE, (c + 1) * ISLICE)
        m = dict(common)
        m["wt"] = np.ascontiguousarray(wT[:, sl])
        m["hs"] = np.ascontiguousarray(h[sl, :])
        in_maps.append(m)
    return in_maps


def kernel(**inputs):
    from concourse.bass_utils import run_bass_kernel_spmd

    if "nc" not in _cache:
        _cache["nc"] = _build_bass()
    nc = _cache["nc"]

    in_maps = _make_in_maps(**inputs)
    res = run_bass_kernel_spmd(nc, in_maps, core_ids=list(range(NCORES)))
    out = np.concatenate([r["out"] for r in res.results], axis=0)
    return np.ascontiguousarray(out, dtype=np.float32)


# revision 14
# speedup vs baseline: 2.7675x; 1.2045x over previous
"""Trainium2 Bass kernel for MultiHeadEdgeAwareMessagePassing.

Math restructure (validated vs reference, ~1e-3 final rel err incl. bf16):
  logits[i,j,h] = s_q[i,h] + s_k[j,h] + w[i,j]*c1[h] + c0[h]   (valid j: w>0)
  alpha = softmax_j(logits) * w
s_q, c0 are constant over j and cancel in the softmax; bk's contribution to
s_k scales numerator and denominator equally and cancels too. With
g[j,h] = exp(h[j]@a_k[h]), a_k[h] = u_k[h] @ Wk[h-block], v = h@Wv^T + bv:
  msg[i,h,:] = Num_h[i,:] / Den_h[i]
  Num_h = W1^T (g_h*v_h)
  Den_h = mask^T g_h + c1_h (W1^T g_h)
where mask=[w>0], W1=relu(w)  (exp(c1 w) ~= 1 + c1 w, |c1 w| < 0.02; the
dropped quadratic term changes the final output by ~3e-6 relative).

Sharding: destination rows i split across 8 cores (384 rows each). Each core
reads its [3072, 384] slice of w^T plus replicated h^T and the small weights.
Host-side transposes are layout prep only; all compute runs on device.
"""

import numpy as np

N = 3072
D = 256
H = 4
DH = 64
DE = 8
NCORES = 8
ISLICE = N // NCORES  # 384
NSUB = ISLICE // 128  # 3
CJT = 4               # j-tiles per chunk
NCH = N // (128 * CJT)  # 6 chunks

_cache = {}


def _build_bass():
    import concourse.bass as bass
    import concourse.tile as tile
    from concourse import bacc, mybir
    from concourse.bass import ts
    from concourse.masks import make_identity

    dt = mybir.dt
    AF = mybir.ActivationFunctionType
    OP = mybir.AluOpType

    nc = bacc.Bacc("TRN2", target_bir_lowering=False, debug=False,
                   num_devices=NCORES)

    wt_d = nc.dram_tensor("wt", [N, ISLICE], dt.float32, kind="ExternalInput")
    ht_d = nc.dram_tensor("ht", [D, N], dt.float32, kind="ExternalInput")
    hs_d = nc.dram_tensor("hs", [ISLICE, D], dt.float32, kind="ExternalInput")
    Wk_d = nc.dram_tensor("Wk", [D, D], dt.float32, kind="ExternalInput")
    WvT_d = nc.dram_tensor("WvT", [D, D], dt.float32, kind="ExternalInput")
    WoT_d = nc.dram_tensor("WoT", [D, D], dt.float32, kind="ExternalInput")
    u_d = nc.dram_tensor("u", [H, 2 * DH + DE], dt.float32, kind="ExternalInput")
    bv_d = nc.dram_tensor("bv", [D], dt.float32, kind="ExternalInput")
    bo_d = nc.dram_tensor("bo", [D], dt.float32, kind="ExternalInput")
    Wew_d = nc.dram_tensor("Wew", [H * DE, 1], dt.float32, kind="ExternalInput")
    gam_d = nc.dram_tensor("gamma", [D], dt.float32, kind="ExternalInput")
    bet_d = nc.dram_tensor("beta", [D], dt.float32, kind="ExternalInput")
    out_d = nc.dram_tensor("out", [ISLICE, D], dt.float32, kind="ExternalOutput")

    bf = dt.bfloat16
    f32 = dt.float32

    with tile.TileContext(nc) as tc:
        with (
            tc.tile_pool(name="consts", bufs=1) as consts,
            tc.tile_pool(name="wtp", bufs=3) as wtp,
            tc.tile_pool(name="htf", bufs=2) as htfp,
            tc.tile_pool(name="elem", bufs=2) as elem,
            tc.tile_pool(name="rhsp", bufs=3) as rhsp,
            tc.tile_pool(name="gp", bufs=2) as gp,
            tc.tile_pool(name="small", bufs=4) as small,
            tc.tile_pool(name="outp", bufs=2) as outp,
            tc.tile_pool(name="acc", bufs=1, space="PSUM") as accp,
            tc.tile_pool(name="pre4", bufs=2, space="PSUM") as pre4,
            tc.tile_pool(name="presk", bufs=1, space="PSUM") as presk,
        ):
            # ------------- small constants (SWDGE, cast to bf16) -------------
            rhs_wv = consts.tile([128, 2, D], bf, tag="rhswv")
            nc.gpsimd.dma_start(rhs_wv,
                                WvT_d.ap().rearrange("(a p) n -> p a n", p=128))
            # Wk regrouped per head: Wk2[d, h, :] = Wk[h*64+d, :]
            Wk2 = consts.tile([DH, H, D], bf, tag="wk2")
            nc.gpsimd.dma_start(
                Wk2, bass.AP(tensor=Wk_d, offset=0,
                             ap=[[D, DH], [DH * D, H], [1, D]]))
            # u4[d, h] = u_k[h, d];  ue4[d, h] = u_e[h, d]
            u4 = consts.tile([DH, H], bf, tag="u4")
            nc.gpsimd.dma_start(
                u4, bass.AP(tensor=u_d, offset=DH,
                            ap=[[1, DH], [2 * DH + DE, H]]))
            ue4 = consts.tile([DE, H], bf, tag="ue4")
            nc.gpsimd.dma_start(
                ue4, bass.AP(tensor=u_d, offset=2 * DH,
                             ap=[[1, DE], [2 * DH + DE, H]]))
            # Wew2[d, h] = We_w[h*8+d, 0]
            Wew2 = consts.tile([DE, H], bf, tag="wew2")
            nc.gpsimd.dma_start(
                Wew2, bass.AP(tensor=Wew_d, offset=0, ap=[[1, DE], [DE, H]]))
            bv_row = consts.tile([1, D], bf, tag="bvrow")
            nc.gpsimd.dma_start(bv_row, bv_d.ap().rearrange("(o f) -> o f", o=1))

            ones_sb = consts.tile([1, 128], bf, tag="ones")
            nc.vector.memset(ones_sb, 1.0)

            # ------------- epilogue constants (SWDGE, off critical path) -----
            WoT_sb = consts.tile([128, 2, D], bf, tag="wot")
            nc.gpsimd.dma_start(WoT_sb,
                                WoT_d.ap().rearrange("(a p) n -> p a n", p=128))
            bo_row = consts.tile([1, 256], bf, tag="borow")
            nc.gpsimd.dma_start(bo_row, bo_d.ap().rearrange("(o f) -> o f", o=1))
            ident = consts.tile([128, 128], bf, tag="ident")
            make_identity(nc, ident)
            gam_sb = consts.tile([128, D], f32, tag="gam")
            nc.gpsimd.dma_start(
                gam_sb, bass.AP(tensor=gam_d, offset=0, ap=[[0, 128], [1, D]]))
            bet_sb = consts.tile([128, D], f32, tag="bet")
            nc.gpsimd.dma_start(
                bet_sb, bass.AP(tensor=bet_d, offset=0, ap=[[0, 128], [1, D]]))
            eps_sb = consts.tile([128, 1], f32, tag="eps")
            nc.vector.memset(eps_sb, 1e-5)

            # ------------- setup matmuls -------------
            # a_k^T[dm, h] = sum_d Wk[h*64+d, dm] u_k[h, d]
            rhs_ak = consts.tile([128, 2, H], bf, tag="rhsak")
            for b in range(2):
                ps_ak = presk.tile([128, H], f32, tag="sk4")
                for h in range(H):
                    nc.tensor.matmul(ps_ak[:, h:h + 1],
                                     Wk2[:, h, 128 * b:128 * (b + 1)],
                                     u4[:, h:h + 1], start=True, stop=True)
                nc.vector.tensor_copy(rhs_ak[:, b, :], ps_ak)

            # c1[h] = sum_d We_w[h*8+d] u_e[h, d], broadcast to partitions
            ps_c1 = presk.tile([1, H], f32, tag="sk4")
            for h in range(H):
                nc.tensor.matmul(ps_c1[:, h:h + 1], Wew2[:, h:h + 1],
                                 ue4[:, h:h + 1], start=True, stop=True)
            c1row = consts.tile([1, H], bf, tag="c1row")
            nc.vector.tensor_copy(c1row, ps_c1)
            ps_c1b = presk.tile([128, H], f32, tag="sk4")
            nc.tensor.matmul(ps_c1b, ones_sb, c1row, start=True, stop=True)
            c1b = consts.tile([128, H], f32, tag="c1b")
            nc.vector.tensor_copy(c1b, ps_c1b)

            # ---------------- persistent accumulators ----------------
            # cols 0:256 = W1.gV, 256:260 = W1.g, 260:264 = mask.g
            psA = [accp.tile([128, 264], f32, tag=f"A{s}", name=f"psA{s}")
                   for s in range(NSUB)]

            ht_sb = consts.tile([128, 2, N], bf, tag="ht")
            ht_re = ht_d.ap().rearrange("(a p) n -> p a n", p=128)

            # ---------------- main loop ----------------
            for ch in range(NCH):
                wt4 = wtp.tile([128, CJT, ISLICE], f32, tag="wt")
                nc.sync.dma_start(
                    wt4, wt_d[ts(ch, 128 * CJT), :].rearrange(
                        "(j p) i -> p j i", p=128))

                # h^T chunk: f32 via HWDGE, cast to bf16 on ACT/DVE alternately
                htf = htfp.tile([128, 2, 128 * CJT], f32, tag="htf")
                nc.sync.dma_start(htf, ht_re[:, :, ts(ch, 128 * CJT)])
                htc = ht_sb[:, :, ts(ch, 128 * CJT)]
                if ch % 2 == 0:
                    nc.scalar.copy(htc, htf)
                else:
                    nc.vector.tensor_copy(htc, htf)

                W1c = elem.tile([128, CJT, ISLICE], bf, tag="W1")
                nc.scalar.activation(W1c, wt4, AF.Relu)
                mskc = elem.tile([128, CJT, ISLICE], bf, tag="msk")
                nc.vector.tensor_scalar(mskc, wt4, 0.0, None, op0=OP.is_gt)

                # --- v and s_k for the CJT j-tiles of this chunk ---
                ps_v4 = pre4.tile([128, CJT, 256], f32, tag="v4")
                ps_sk4 = presk.tile([128, CJT, H], f32, tag="sk4")
                for jm in range(CJT):
                    jt = ch * CJT + jm
                    for a in range(2):
                        nc.tensor.matmul(ps_v4[:, jm, :],
                                         ht_sb[:, a, ts(jt, 128)],
                                         rhs_wv[:, a, :],
                                         start=(a == 0), stop=False)
                        nc.tensor.matmul(ps_sk4[:, jm, :],
                                         ht_sb[:, a, ts(jt, 128)],
                                         rhs_ak[:, a, :],
                                         start=(a == 0), stop=(a == 1))
                    nc.tensor.matmul(ps_v4[:, jm, :], ones_sb, bv_row,
                                     start=False, stop=True)

                g32 = gp.tile([128, CJT, H], f32, tag="g32")
                nc.scalar.activation(g32, ps_sk4, AF.Exp)

                rhs4 = rhsp.tile([128, CJT, 260], bf, tag="rhsbig")
                g32b = bass.AP(tensor=g32.tensor, offset=g32.offset,
                               ap=[g32.ap[0], g32.ap[1], g32.ap[2], [0, DH]])
                nc.vector.tensor_tensor(
                    out=rhs4[:, :, 0:256].rearrange(
                        "p j (h d) -> p j h d", h=H),
                    in0=ps_v4.rearrange("p j (h d) -> p j h d", h=H),
                    in1=g32b, op=OP.mult)
                nc.vector.tensor_copy(rhs4[:, :, 256:260], g32)

                st = (ch == 0)
                sp = (ch == NCH - 1)
                for jm in range(CJT):
                    for s in range(NSUB):
                        sl = ts(s, 128)
                        nc.tensor.matmul(psA[s][:, 0:260], W1c[:, jm, sl],
                                         rhs4[:, jm, :], start=st, stop=sp,
                                         skip_group_check=True)
                        nc.tensor.matmul(psA[s][:, 260:264], mskc[:, jm, sl],
                                         rhs4[:, jm, 256:260], start=st, stop=sp,
                                         skip_group_check=True)

            # ---------------- epilogue ----------------
            for s in range(NSUB):
                dg = small.tile([128, H], f32, tag="dg")
                nc.vector.tensor_copy(dg, psA[s][:, 256:260])
                den = small.tile([128, H], f32, tag="den")
                nc.vector.tensor_mul(den, c1b, dg)
                nc.vector.tensor_add(den, den, psA[s][:, 260:264])
                rden = small.tile([128, H], f32, tag="rden")
                nc.vector.reciprocal(rden, den)

                msg = outp.tile([128, D], bf, tag="msg")
                for h in range(H):
                    hsl = slice(h * DH, (h + 1) * DH)
                    nc.vector.tensor_scalar(msg[:, hsl], psA[s][:, hsl],
                                            rden[:, h:h + 1], None, op0=OP.mult)

                msgT = outp.tile([128, 2, 128], bf, tag="msgT")
                for b in range(2):
                    ps_t = pre4.tile([128, 128], bf, tag="v4")
                    nc.tensor.transpose(ps_t, msg[:, ts(b, 128)], ident)
                    nc.vector.tensor_copy(msgT[:, b, :], ps_t)

                ps_o = pre4.tile([128, D], f32, tag="v4")
                nc.tensor.matmul(ps_o, msgT[:, 0, :], WoT_sb[:, 0, :],
                                 start=True, stop=False)
                nc.tensor.matmul(ps_o, msgT[:, 1, :], WoT_sb[:, 1, :],
                                 start=False, stop=False)
                nc.tensor.matmul(ps_o, ones_sb, bo_row, start=False, stop=True)

                x = outp.tile([128, D], f32, tag="x")
                hseg = outp.tile([128, D], f32, tag="hseg")
                nc.sync.dma_start(hseg, hs_d[ts(s, 128), :])
                nc.vector.tensor_add(x, ps_o, hseg)

                stats = small.tile([128, 6], f32, tag="stats")
                nc.vector.bn_stats(out=stats, in_=x)
                mv = small.tile([128, 2], f32, tag="mv")
                nc.vector.bn_aggr(out=mv, in_=stats)
                sd = small.tile([128, 1], f32, tag="sd")
                nc.scalar.activation(sd, mv[:, 1:2], AF.Sqrt, bias=eps_sb)
                rstd = small.tile([128, 1], f32, tag="rstd")
                nc.vector.reciprocal(rstd, sd)

                y = outp.tile([128, D], f32, tag="y")
                nc.vector.tensor_scalar(y, x, mv[:, 0:1], rstd,
                                        op0=OP.subtract, op1=OP.mult)
                ot = outp.tile([128, D], f32, tag="ot")
                nc.vector.tensor_mul(ot, y, gam_sb)
                nc.vector.tensor_add(ot, ot, bet_sb)
                nc.sync.dma_start(out_d[ts(s, 128), :], ot)

    nc.compile()
    return nc


def _make_in_maps(h, w, Wk, Wv, bv, We_w, u, Wo, bo, gamma, beta, **_unused):
    f = np.float32
    h = np.ascontiguousarray(h, dtype=f)
    wT = np.ascontiguousarray(np.asarray(w, dtype=f).T)
    common = {
        "ht": np.ascontiguousarray(h.T),
        "Wk": np.ascontiguousarray(Wk, dtype=f),
        "WvT": np.ascontiguousarray(np.asarray(Wv, dtype=f).T),
        "WoT": np.ascontiguousarray(np.asarray(Wo, dtype=f).T),
        "u": np.ascontiguousarray(u, dtype=f),
        "bv": np.ascontiguousarray(bv, dtype=f),
        "bo": np.ascontiguousarray(bo, dtype=f),
        "Wew": np.ascontiguousarray(We_w, dtype=f),
        "gamma": np.ascontiguousarray(gamma, dtype=f),
        "beta": np.ascontiguousarray(beta, dtype=f),
    }
    in_maps = []
    for c in range(NCORES):
        sl = slice(c * ISLICE, (c + 1) * ISLICE)
        m = dict(common)
        m["wt"] = np.ascontiguousarray(wT[:, sl])
        m["hs"] = np.ascontiguousarray(h[sl, :])
        in_maps.append(m)
    return in_maps


def kernel(**inputs):
    from concourse.bass_utils import run_bass_kernel_spmd

    if "nc" not in _cache:
        _cache["nc"] = _build_bass()
    nc = _cache["nc"]

    in_maps = _make_in_maps(**inputs)
    res = run_bass_kernel_spmd(nc, in_maps, core_ids=list(range(NCORES)))
    out = np.concatenate([r["out"] for r in res.results], axis=0)
    return np.ascontiguousarray(out, dtype=np.float32)
